# revision 46
# baseline (speedup 1.0000x reference)
"""Trainium2 Bass kernel for nn_AttentionHead: causal attention head.

reference:
    scores = (E @ qk) @ E.T           # [N, N],  E: [4096, 2048]
    scores += causal_mask (strict upper = -inf)
    attn = softmax(scores, axis=-1)
    out = (attn @ E) @ ov             # [4096, 2048]

Strategy (8 NeuronCores, SPMD, no collectives):
  - Each core owns 4 query tiles of 128 rows, one per causal "band":
    core i owns global q-tiles {C*(B-1-t)+i : t in 0..B-1}, with key extents
    {128*C*(B-t)} = {4096, 3072, 2048, 1024}. Identical work on every core ->
    a single uniform instruction graph; only input DATA differs per core.
    (This cyclic assignment provably minimizes the per-slot max extent, the
    binding constraint for a uniform SPMD instruction graph.)
  - The whole pipeline runs in plain fp16 (PE upconverts fp16 exactly to its
    internal FP22, accumulates fp32). Measured final rel err 6.8e-3 vs the
    2e-2 gate: softmax scores are O(1000) but near-one-hot, so fp16-level
    score error (~0.5 rms) only perturbs the handful of near-tie rows.
    The fp16 hi stationary operand for the score matmul is pre-scaled by 2^8
    (exact in fp16) purely to keep the same PSUM/exp scaling as earlier fp8
    variants; exp descales via scale=1/256.
  - Softmax rows live on partitions ([q, j] layout): reduce_max / exp-with-
    bias / accum_out are all native per-partition ops. P tiles are then
    PE-transposed (128x128) so the PV matmul can contract over j.
  - Host prep is layout/dtype only: fp16 casts, transposes, tiling;
    partition-major layouts for fat contiguous DMA runs; wq weights arrive
    in d-tile pairs (one DMA trigger loads two d-tiles, halving trigger
    count / semaphore pressure -- triggers cost ~0.7us each and ~8 DMA
    semaphores alias across queues).
  - Output is stored fp16 (upcast on host): halves the tail out-DMA.

Dataflow per core (D=2048, DP=16 d-tiles, JW=512):
  Q^T[d',q]  = sum_d qk[d,d'] * EownT[d,q]         (256 fp16 MMs, N=512)
  S[q,j]     = sum_d' Q^T[d',q] * ET[d',j]         (320 fp16 MMs, N=512)
  P = exp((S256 - rowmax256)/256)                  (ACT, fp16 out, rowsum via accum_out)
  P^T tiles via PE transpose                       (80 transposes)
  OpT[d,q]   = sum_j E[j,d] * P^T[j,q]             (512 MMs fp16)
  out[q,d2]  = (sum_d OpT[d,q] * ov[d,d2]) / rowsum  (256 MMs fp16, N=512)
"""
import sys

for _p in ('/opt/trn_rl_repo', '/opt/pypackages'):
    if _p not in sys.path:
        sys.path.insert(0, _p)

import numpy as np

# ---- configuration (hardcoded for the graded problem) ----
N_CTX = 4096
D_MODEL = 2048
N_CORES = 8
B_BANDS = 4
QT = 128                       # q-tile rows

MASK_NEG = -1e30
LSCALE = 256.0                 # 2^8 scale on the score path (exact in fp16)


def build_program(C=N_CORES, B=B_BANDS, D=D_MODEL):
    import concourse.bass as bass
    import concourse.mybir as mybir
    from concourse import bacc, tile
    from concourse.masks import make_identity

    F32 = mybir.dt.float32
    F16 = mybir.dt.float16

    N = C * B * QT                 # total context
    NQ = B * QT                    # rows per core
    DP = D // 128                  # d tiles
    JW = min(512, QT * C)          # j / free-dim window
    NW = N // JW                   # S windows over full context
    NJT = N // 128                 # j tiles
    MASKW = QT * C                 # mask window width (last cols of each extent)
    NDC = D // JW                  # output d2 chunks

    exts = [QT * C * (B - t) for t in range(B)]   # extent per local q-tile t

    def n_jt(jt):                  # active moving width at j-tile jt
        return 128 * (B - jt // C)

    nc = bacc.Bacc("TRN2", target_bir_lowering=False, debug=False)

    # inputs (pre-tiled on host for contiguous DMA; qrth partition-major)
    qrth_d = nc.dram_tensor("qrth", [128, DP, NQ], F16, kind="ExternalInput")
    # wq in pairs of output d-tiles (dim "two"): one trigger = two d-tiles
    wqh_d = nc.dram_tensor("wqh", [DP // 2, 128, 2, DP, 128], F16,
                           kind="ExternalInput")
    eth_d = nc.dram_tensor("eth", [NW, 128, DP, JW], F16, kind="ExternalInput")
    ev_d = nc.dram_tensor("ev", [DP, 128, NJT, 128], F16, kind="ExternalInput")
    ov_d = nc.dram_tensor("ov", [NDC, 128, DP, JW], F16, kind="ExternalInput")
    mask_d = nc.dram_tensor("mask", [128, MASKW], F32, kind="ExternalInput")
    # fp16 output store (upcast on host)
    out_d = nc.dram_tensor("out", [NQ, D], F16, kind="ExternalOutput")

    with tile.TileContext(nc) as tc:
        with (
            tc.tile_pool(name="const", bufs=1) as constp,
            tc.tile_pool(name="qt", bufs=1) as qtp,
            tc.tile_pool(name="pt", bufs=1) as ptp,
            tc.tile_pool(name="small", bufs=1) as smallp,
            tc.tile_pool(name="mm_ps", bufs=4, space="PSUM") as mmps,
            tc.tile_pool(name="tr_ps", bufs=2, space="PSUM") as trps,
            tc.tile_pool(name="pv_ps", bufs=2, space="PSUM") as pvps,
        ):
            ident = constp.tile([128, 128], F16, tag="ident")
            make_identity(nc, ident[:])
            # PE clock warm-up: the tensor engine p-state ramps with sustained
            # activity, and the PE otherwise idles ~10us at kernel start
            # waiting for the first DMAs. Throwaway identity transposes (no
            # data deps, existing PSUM pool slots) keep the PE busy through
            # that window so the first real matmuls run at full clock.
            for _wu in range(80):
                trp = trps.tile([128, 128], F16, tag="tr")
                nc.tensor.transpose(trp[:], ident[:], ident[:])
            # mask load is issued later (after the Q-phase prologue DMAs) to
            # keep the first matmul's deps at the head of the DMA queues
            mask_sb = constp.tile([128, MASKW], F32, tag="mask")

            # PT[jt]: transposed attention weights, [j-part, q-cols prefix]
            pt = [ptp.tile([128, n_jt(jt)], F16, tag=f"pt{jt}", name=f"pt{jt}") for jt in range(NJT)]

            # S-phase stationary: qth = fp16(256*Q^T)
            qth = [qtp.tile([128, NQ], F16, tag=f"qth{dp}", name=f"qth{dp}") for dp in range(DP)]

            negmax = [smallp.tile([128, 1], F32, tag=f"ngm{t}", name=f"ngm{t}") for t in range(B)]
            rsum = [smallp.tile([128, 1], F32, tag=f"rs{t}", name=f"rs{t}") for t in range(B)]
            recip = [smallp.tile([128, 1], F32, tag=f"rc{t}", name=f"rc{t}") for t in range(B)]
            rspart = {}
            rmax = [smallp.tile([128, NW], F32, tag=f"rmx{t}", name=f"rmx{t}")
                    for t in range(B)]

            # ev pool opens before ew (proper nesting: released after PV)
            # so value tiles can prefetch during mid-S on the scalar DMA
            # queue, not head-blocking ew loads
            evp_cm = tc.tile_pool(name="evs", bufs=5)
            evp = evp_cm.__enter__()
            ev_tiles = {}

            def load_ev(dt):
                evs = evp.tile([128, NJT, 128], F16, tag="evs", name="evs")
                nc.scalar.dma_start(evs[:], ev_d[dt])
                ev_tiles[dt] = evs

            # ew pool: 3 bufs, loaded two windows ahead, so the thin tail
            # windows (1 q-tile of PE work each) can hide their 2MB loads
            ewp_cm = tc.tile_pool(name="ew", bufs=3)
            ewp = ewp_cm.__enter__()
            ew_tiles = {}

            def load_window(w, eng=None):
                # Q-phase prefetches ride the scalar queue (idle once qrt is
                # in) so they don't delay wq pair loads on sync; S-phase
                # loads use sync (idle during S)
                ewh = ewp.tile([128, DP, JW], F16, tag="ewh", name="ewh")
                (eng or nc.sync).dma_start(ewh[:], eth_d[w])
                ew_tiles[w] = ewh

            # ---------------- Phase Q: 256*Q^T = (256*qk)^T-contracted rows
            with (
                tc.tile_pool(name="qrt", bufs=1) as qrtp,
                tc.tile_pool(name="wq", bufs=3) as wqp,
            ):
                wq_tiles = {}

                def load_wq2(g):
                    # loads output d-tiles 2g and 2g+1 in one trigger
                    wqh_sl = wqp.tile([128, 2, DP, 128], F16, tag="wqh", name="wqh")
                    nc.sync.dma_start(wqh_sl[:], wqh_d[g])
                    wq_tiles[g] = wqh_sl

                qrt_h = qrtp.tile([128, DP, NQ], F16, tag="qrh", name="qrh")
                # DMA triggers cost ~0.7us each on the issuing engine and the
                # first ~6 get dedicated semaphores, so the prologue uses few,
                # fat, partition-major transfers ordered by first use: wq0's
                # first half + early qrt chunks gate the first matmuls.
                # qrt stream on the (idle) scalar engine's DMA queue, wq
                # stream on sync: triggers issue in parallel, no FIFO
                # head-blocking between the two streams
                wqh_sl0 = wqp.tile([128, 2, DP, 128], F16, tag="wqh", name="wqh")
                wq_tiles[0] = wqh_sl0
                nc.sync.dma_start(wqh_sl0[:, 0, 0:DP // 2, :],
                                  wqh_d[0][:, 0, 0:DP // 2, :])
                nc.scalar.dma_start(qrt_h[:, 0:4, :], qrth_d[:, 0:4, :])
                nc.scalar.dma_start(qrt_h[:, 4:8, :], qrth_d[:, 4:8, :])
                nc.sync.dma_start(wqh_sl0[:, 0, DP // 2:, :],
                                  wqh_d[0][:, 0, DP // 2:, :])
                nc.scalar.dma_start(qrt_h[:, 8:12, :], qrth_d[:, 8:12, :])
                nc.scalar.dma_start(qrt_h[:, 12:DP, :], qrth_d[:, 12:DP, :])
                nc.sync.dma_start(wqh_sl0[:, 1], wqh_d[0][:, 1])
                load_wq2(1)
                load_wq2(2)
                nc.sync.dma_start(mask_sb[:], mask_d[:])

                for dp in range(DP):
                    wqh_sl = wq_tiles[dp // 2]
                    h = dp % 2
                    ps = mmps.tile([128, NQ], F32, tag="mm")
                    for dk in range(DP):
                        nc.tensor.matmul(ps[:], wqh_sl[:, h, dk], qrt_h[:, dk, :],
                                         start=(dk == 0), stop=(dk == DP - 1))
                    if h == 1:
                        wq_tiles.pop(dp // 2)
                    if h == 0 and dp // 2 + 3 < DP // 2:
                        load_wq2(dp // 2 + 3)
                    # window prefetch AFTER the last wq trigger (dp==11's
                    # load_wq2(7... is issued at dp==10) so the 2MB window
                    # transfers don't delay wq data the PE needs sooner;
                    # w0/w1 still land well before S starts (~67us)
                    if dp == 11:
                        load_window(0)
                    elif dp == 13:
                        load_window(1)
                    nc.vector.tensor_copy(qth[dp][:], ps[:])

            # ---------------- Phase S: scores + softmax + P^T
            with (
                tc.tile_pool(name="s", bufs=1) as sp,
                tc.tile_pool(name="p", bufs=2) as pp,
            ):
                s_t = [sp.tile([128, exts[t]], F32, tag=f"s{t}", name=f"s{t}") for t in range(B)]

                def softmax_t(t):
                    ext = exts[t]
                    nc.vector.reduce_max(
                        out=negmax[t][:], in_=rmax[t][:, :ext // JW],
                        axis=mybir.AxisListType.X, negate=True)
                    # descale: bias for exp must be -rowmax = -rowmax256/256
                    nc.vector.tensor_scalar_mul(
                        negmax[t][:], negmax[t][:], 1.0 / LSCALE)
                    for w2 in range(ext // JW):
                        pwin = pp.tile([128, JW], F16, tag=f"p{t}", name=f"p{t}")
                        rp = smallp.tile([128, 1], F32, tag=f"rsp{t}_{w2}", name=f"rsp{t}_{w2}")
                        rspart[(t, w2)] = rp
                        nc.scalar.activation(
                            pwin[:], s_t[t][:, w2 * JW:(w2 + 1) * JW],
                            mybir.ActivationFunctionType.Exp,
                            bias=negmax[t][:], scale=1.0 / LSCALE, accum_out=rp[:])
                        for jj in range(JW // 128):
                            jt = w2 * (JW // 128) + jj
                            trp = trps.tile([128, 128], F16, tag="tr")
                            nc.tensor.transpose(
                                trp[:], pwin[:, jj * 128:(jj + 1) * 128], ident[:])
                            nc.vector.tensor_copy(
                                pt[jt][:, t * 128:(t + 1) * 128], trp[:])
                    # rowsum = sum of window partials; recip
                    nc.vector.tensor_copy(rsum[t][:], rspart[(t, 0)][:])
                    for w2 in range(1, ext // JW):
                        nc.vector.tensor_add(
                            rsum[t][:], rsum[t][:], rspart[(t, w2)][:])
                    nc.vector.reciprocal(recip[t][:], rsum[t][:])

                for w in range(NW):
                    if w + 2 < NW:
                        load_window(w + 2)
                    # ev value-tile prefetch spread over mid-S iterations so
                    # it does not collide with the tail windows' loads
                    if 2 <= w <= 6:
                        load_ev(w - 2)
                    ewh = ew_tiles.pop(w)
                    for t in range(B):
                        if exts[t] <= JW * w:
                            continue
                        ps = mmps.tile([128, JW], F32, tag="mm")
                        for dp in range(DP):
                            nc.tensor.matmul(ps[:], qth[dp][:, t * 128:(t + 1) * 128],
                                             ewh[:, dp],
                                             start=(dp == 0), stop=(dp == DP - 1))
                        # copy scores to SBUF, folding in the causal mask on
                        # the last MASKW columns; track per-window row max
                        nmw = MASKW // JW
                        wloc = exts[t] // JW - 1 - w   # windows from the end
                        if wloc < nmw:
                            moff = (nmw - 1 - wloc) * JW
                            nc.vector.tensor_add(
                                s_t[t][:, w * JW:(w + 1) * JW], ps[:],
                                mask_sb[:, moff:moff + JW])
                        else:
                            nc.vector.tensor_copy(
                                s_t[t][:, w * JW:(w + 1) * JW], ps[:])
                        nc.vector.reduce_max(
                            out=rmax[t][:, w:w + 1],
                            in_=s_t[t][:, w * JW:(w + 1) * JW],
                            axis=mybir.AxisListType.X)
                        if JW * (w + 1) == exts[t]:
                            softmax_t(t)

            ewp_cm.__exit__(None, None, None)

            # ---------------- Phase PV: OpT[d, q] = sum_j E[j,d] P^T[j,q]
            with (
                tc.tile_pool(name="opt", bufs=1) as optp,
                tc.tile_pool(name="ovs", bufs=2) as ovp,
                tc.tile_pool(name="osb", bufs=2) as osbp,
            ):
                ov_tiles = {}

                def load_ov(dc):
                    ovs = ovp.tile([128, DP, JW], F16, tag="ovs", name="ovs")
                    nc.scalar.dma_start(ovs[:], ov_d[dc])
                    ov_tiles[dc] = ovs

                opt = [optp.tile([128, NQ], F16, tag=f"opt{dt}", name=f"opt{dt}") for dt in range(DP)]
                NPRE = min(5, DP)   # tiles 0..4 already prefetched mid-S
                for dt in range(DP):
                    evs = ev_tiles.pop(dt)
                    ps = pvps.tile([128, NQ], F32, tag="pv")
                    for jt in range(NJT):
                        nw_ = n_jt(jt)
                        nc.tensor.matmul(ps[:, :nw_], evs[:, jt], pt[jt][:, :nw_],
                                         start=(jt == 0), stop=(jt == NJT - 1))
                    if dt + NPRE < DP:
                        load_ev(dt + NPRE)
                    elif dt == max(0, DP - NPRE):
                        load_ov(0)
                    elif dt == max(1, DP - NPRE + 1):
                        load_ov(1)
                    nc.vector.tensor_copy(opt[dt][:], ps[:])

                # ---------------- Phase O: out = (OpT^T @ ov) * recip
                if True:
                    for dc in range(NDC):
                        if dc + 2 < NDC:
                            load_ov(dc + 2)
                        ovs = ov_tiles.pop(dc)
                        for t in range(B):
                            ps = mmps.tile([128, JW], F32, tag="mm")
                            for dt in range(DP):
                                nc.tensor.matmul(
                                    ps[:], opt[dt][:, t * 128:(t + 1) * 128],
                                    ovs[:, dt],
                                    start=(dt == 0), stop=(dt == DP - 1))
                            osb = osbp.tile([128, JW], F16, tag="osb")
                            nc.vector.tensor_scalar_mul(osb[:], ps[:], recip[t][:])
                            nc.sync.dma_start(
                                out_d[t * 128:(t + 1) * 128,
                                      dc * JW:(dc + 1) * JW], osb[:])

            evp_cm.__exit__(None, None, None)

    nc.compile()
    return nc


def make_in_maps(embedding, qk, ov, C=N_CORES, B=B_BANDS):
    """Host-side layout/dtype prep. Returns (in_maps, gtiles_per_core)."""
    N, D = embedding.shape
    DP = D // 128
    DPP = DP // 2
    JW = min(512, QT * C)
    NW = N // JW
    NJT = N // 128
    NQ = B * QT
    NDC = D // JW
    MASKW = QT * C

    E = np.ascontiguousarray(embedding.astype(np.float32))
    ET = np.ascontiguousarray(E.T)
    Eh = E.astype(np.float16)
    ETh = np.ascontiguousarray(ET.astype(np.float16))
    W = qk.astype(np.float32)
    Wh = W.astype(np.float16)
    # 256*Wh is exact in fp16 (|W| < 1 so |256*Wh| < 65504)
    Wh256 = (LSCALE * Wh.astype(np.float32)).astype(np.float16)
    OVh = ov.astype(np.float16)

    eth_t = np.ascontiguousarray(
        ETh.reshape(DP, 128, NW, JW).transpose(2, 1, 0, 3))
    # paired layout: wqh[c2, b, j, a, d] = 256*Wh[a*128+b, (2*c2+j)*128+d]
    wqh_t = np.ascontiguousarray(
        Wh256.reshape(DP, 128, DPP, 2, 128).transpose(2, 1, 3, 0, 4))
    ev_t = np.ascontiguousarray(
        Eh.reshape(NJT, 128, DP, 128).transpose(2, 1, 0, 3))
    ov_t = np.ascontiguousarray(
        OVh.reshape(DP, 128, NDC, JW).transpose(2, 1, 0, 3))

    r = np.arange(128)[:, None]
    m = np.arange(MASKW)[None, :]

    in_maps = []
    gtiles_all = []
    for i in range(C):
        gtiles = [C * (B - 1 - t) + i for t in range(B)]
        gtiles_all.append(gtiles)
        qrh = np.concatenate(
            [ETh[:, 128 * g:128 * (g + 1)] for g in gtiles], axis=1)
        # partition-major for fat contiguous DMA runs:
        # qrth[b, dk, q] = ETh_own[dk*128+b, q]
        qrth = qrh.reshape(DP, 128, NQ).transpose(1, 0, 2)
        mask = np.where(m <= 128 * i + r, 0.0, MASK_NEG).astype(np.float32)
        in_maps.append({
            "qrth": np.ascontiguousarray(qrth),
            "wqh": wqh_t,
            "eth": eth_t,
            "ev": ev_t, "ov": ov_t,
            "mask": mask,
        })
    return in_maps, gtiles_all


_CACHED = {}


def kernel(embedding, qk, ov):
    from concourse.bass_utils import run_bass_kernel_spmd

    key = "main"
    if key not in _CACHED:
        _CACHED[key] = build_program()
    nc = _CACHED[key]

    in_maps, gtiles_all = make_in_maps(embedding, qk, ov)
    res = run_bass_kernel_spmd(nc, in_maps, core_ids=list(range(N_CORES)))

    N, D = embedding.shape
    out = np.empty((N, D), dtype=np.float32)
    for i in range(N_CORES):
        o = np.asarray(res.results[i]["out"], dtype=np.float32)
        for t, g in enumerate(gtiles_all[i]):
            out[128 * g:128 * (g + 1)] = o[128 * t:128 * (t + 1)]
    return out


# revision 47
# speedup vs baseline: 1.0164x; 1.0164x over previous
"""Trainium2 Bass kernel for nn_AttentionHead: causal attention head.

reference:
    scores = (E @ qk) @ E.T           # [N, N],  E: [4096, 2048]
    scores += causal_mask (strict upper = -inf)
    attn = softmax(scores, axis=-1)
    out = (attn @ E) @ ov             # [4096, 2048]

Strategy (8 NeuronCores, SPMD, no collectives):
  - Each core owns 4 query tiles of 128 rows, one per causal "band":
    core i owns global q-tiles {C*(B-1-t)+i : t in 0..B-1}, with key extents
    {128*C*(B-t)} = {4096, 3072, 2048, 1024}. Identical work on every core ->
    a single uniform instruction graph; only input DATA differs per core.
    (This cyclic assignment provably minimizes the per-slot max extent, the
    binding constraint for a uniform SPMD instruction graph.)
  - The whole pipeline runs in plain fp16 (PE upconverts fp16 exactly to its
    internal FP22, accumulates fp32). Measured final rel err 6.8e-3 vs the
    2e-2 gate: softmax scores are O(1000) but near-one-hot, so fp16-level
    score error (~0.5 rms) only perturbs the handful of near-tie rows.
    The fp16 hi stationary operand for the score matmul is pre-scaled by 2^8
    (exact in fp16) purely to keep the same PSUM/exp scaling as earlier fp8
    variants; exp descales via scale=1/256.
  - Softmax rows live on partitions ([q, j] layout): reduce_max / exp-with-
    bias / accum_out are all native per-partition ops. P tiles are then
    PE-transposed (128x128) so the PV matmul can contract over j.
  - Host prep is layout/dtype only: fp16 casts, transposes, tiling;
    partition-major layouts for fat contiguous DMA runs; wq weights arrive
    in d-tile pairs (one DMA trigger loads two d-tiles, halving trigger
    count / semaphore pressure -- triggers cost ~0.7us each and ~8 DMA
    semaphores alias across queues).
  - Output is stored fp16 (upcast on host): halves the tail out-DMA.

Dataflow per core (D=2048, DP=16 d-tiles, JW=512):
  Q^T[d',q]  = sum_d qk[d,d'] * EownT[d,q]         (256 fp16 MMs, N=512)
  S[q,j]     = sum_d' Q^T[d',q] * ET[d',j]         (320 fp16 MMs, N=512)
  P = exp((S256 - rowmax256)/256)                  (ACT, fp16 out, rowsum via accum_out)
  P^T tiles via PE transpose                       (80 transposes)
  OpT[d,q]   = sum_j E[j,d] * P^T[j,q]             (512 MMs fp16)
  out[q,d2]  = (sum_d OpT[d,q] * ov[d,d2]) / rowsum  (256 MMs fp16, N=512)
"""
import sys

for _p in ('/opt/trn_rl_repo', '/opt/pypackages'):
    if _p not in sys.path:
        sys.path.insert(0, _p)

import numpy as np

# ---- configuration (hardcoded for the graded problem) ----
N_CTX = 4096
D_MODEL = 2048
N_CORES = 8
B_BANDS = 4
QT = 128                       # q-tile rows

MASK_NEG = -1e30
LSCALE = 256.0                 # 2^8 scale on the score path (exact in fp16)


def build_program(C=N_CORES, B=B_BANDS, D=D_MODEL):
    import concourse.bass as bass
    import concourse.mybir as mybir
    from concourse import bacc, tile
    from concourse.masks import make_identity

    F32 = mybir.dt.float32
    F16 = mybir.dt.float16

    N = C * B * QT                 # total context
    NQ = B * QT                    # rows per core
    DP = D // 128                  # d tiles
    JW = min(512, QT * C)          # j / free-dim window
    NW = N // JW                   # S windows over full context
    NJT = N // 128                 # j tiles
    MASKW = QT * C                 # mask window width (last cols of each extent)
    NDC = D // JW                  # output d2 chunks

    exts = [QT * C * (B - t) for t in range(B)]   # extent per local q-tile t

    def n_jt(jt):                  # active moving width at j-tile jt
        return 128 * (B - jt // C)

    nc = bacc.Bacc("TRN2", target_bir_lowering=False, debug=False)

    # inputs (pre-tiled on host for contiguous DMA; qrth partition-major)
    qrth_d = nc.dram_tensor("qrth", [128, DP, NQ], F16, kind="ExternalInput")
    # wq in pairs of output d-tiles (dim "two"): one trigger = two d-tiles
    wqh_d = nc.dram_tensor("wqh", [DP // 2, 128, 2, DP, 128], F16,
                           kind="ExternalInput")
    eth_d = nc.dram_tensor("eth", [NW, 128, DP, JW], F16, kind="ExternalInput")
    ev_d = nc.dram_tensor("ev", [DP, 128, NJT, 128], F16, kind="ExternalInput")
    ov_d = nc.dram_tensor("ov", [NDC, 128, DP, JW], F16, kind="ExternalInput")
    mask_d = nc.dram_tensor("mask", [128, MASKW], F32, kind="ExternalInput")
    # fp16 output store (upcast on host)
    out_d = nc.dram_tensor("out", [NQ, D], F16, kind="ExternalOutput")

    with tile.TileContext(nc) as tc:
        with (
            tc.tile_pool(name="const", bufs=1) as constp,
            tc.tile_pool(name="qt", bufs=1) as qtp,
            tc.tile_pool(name="pt", bufs=1) as ptp,
            tc.tile_pool(name="small", bufs=1) as smallp,
            tc.tile_pool(name="mm_ps", bufs=4, space="PSUM") as mmps,
            tc.tile_pool(name="tr_ps", bufs=2, space="PSUM") as trps,
            tc.tile_pool(name="pv_ps", bufs=2, space="PSUM") as pvps,
        ):
            ident = constp.tile([128, 128], F16, tag="ident")
            make_identity(nc, ident[:])
            # mask load is issued later (after the Q-phase prologue DMAs) to
            # keep the first matmul's deps at the head of the DMA queues
            mask_sb = constp.tile([128, MASKW], F32, tag="mask")

            # PT[jt]: transposed attention weights, [j-part, q-cols prefix]
            pt = [ptp.tile([128, n_jt(jt)], F16, tag=f"pt{jt}", name=f"pt{jt}") for jt in range(NJT)]

            # S-phase stationary: qth = fp16(256*Q^T)
            qth = [qtp.tile([128, NQ], F16, tag=f"qth{dp}", name=f"qth{dp}") for dp in range(DP)]

            negmax = [smallp.tile([128, 1], F32, tag=f"ngm{t}", name=f"ngm{t}") for t in range(B)]
            rsum = [smallp.tile([128, 1], F32, tag=f"rs{t}", name=f"rs{t}") for t in range(B)]
            recip = [smallp.tile([128, 1], F32, tag=f"rc{t}", name=f"rc{t}") for t in range(B)]
            rspart = {}
            rmax = [smallp.tile([128, NW], F32, tag=f"rmx{t}", name=f"rmx{t}")
                    for t in range(B)]

            # ev pool opens before ew (proper nesting: released after PV)
            # so value tiles can prefetch during mid-S on the scalar DMA
            # queue, not head-blocking ew loads
            evp_cm = tc.tile_pool(name="evs", bufs=5)
            evp = evp_cm.__enter__()
            ev_tiles = {}

            def load_ev(dt):
                evs = evp.tile([128, NJT, 128], F16, tag="evs", name="evs")
                nc.scalar.dma_start(evs[:], ev_d[dt])
                ev_tiles[dt] = evs

            # ew pool: 3 bufs, loaded two windows ahead, so the thin tail
            # windows (1 q-tile of PE work each) can hide their 2MB loads
            ewp_cm = tc.tile_pool(name="ew", bufs=3)
            ewp = ewp_cm.__enter__()
            ew_tiles = {}

            def load_window(w, eng=None):
                # Q-phase prefetches ride the scalar queue (idle once qrt is
                # in) so they don't delay wq pair loads on sync; S-phase
                # loads use sync (idle during S)
                ewh = ewp.tile([128, DP, JW], F16, tag="ewh", name="ewh")
                (eng or nc.sync).dma_start(ewh[:], eth_d[w])
                ew_tiles[w] = ewh

            # ---------------- Phase Q: 256*Q^T = (256*qk)^T-contracted rows
            with (
                tc.tile_pool(name="qrt", bufs=1) as qrtp,
                tc.tile_pool(name="wq", bufs=3) as wqp,
            ):
                wq_tiles = {}

                def load_wq2(g):
                    # loads output d-tiles 2g and 2g+1 in one trigger
                    wqh_sl = wqp.tile([128, 2, DP, 128], F16, tag="wqh", name="wqh")
                    nc.sync.dma_start(wqh_sl[:], wqh_d[g])
                    wq_tiles[g] = wqh_sl

                qrt_h = qrtp.tile([128, DP, NQ], F16, tag="qrh", name="qrh")
                # DMA triggers cost ~0.7us each on the issuing engine and the
                # first ~6 get dedicated semaphores, so the prologue uses few,
                # fat, partition-major transfers ordered by first use: wq0's
                # first half + early qrt chunks gate the first matmuls.
                # qrt stream on the (idle) scalar engine's DMA queue, wq
                # stream on sync: triggers issue in parallel, no FIFO
                # head-blocking between the two streams
                wqh_sl0 = wqp.tile([128, 2, DP, 128], F16, tag="wqh", name="wqh")
                wq_tiles[0] = wqh_sl0
                nc.sync.dma_start(wqh_sl0[:, 0, 0:DP // 2, :],
                                  wqh_d[0][:, 0, 0:DP // 2, :])
                nc.scalar.dma_start(qrt_h[:, 0:4, :], qrth_d[:, 0:4, :])
                nc.scalar.dma_start(qrt_h[:, 4:8, :], qrth_d[:, 4:8, :])
                nc.sync.dma_start(wqh_sl0[:, 0, DP // 2:, :],
                                  wqh_d[0][:, 0, DP // 2:, :])
                nc.scalar.dma_start(qrt_h[:, 8:12, :], qrth_d[:, 8:12, :])
                nc.scalar.dma_start(qrt_h[:, 12:DP, :], qrth_d[:, 12:DP, :])
                nc.sync.dma_start(wqh_sl0[:, 1], wqh_d[0][:, 1])
                load_wq2(1)
                load_wq2(2)
                nc.sync.dma_start(mask_sb[:], mask_d[:])

                for dp in range(DP):
                    wqh_sl = wq_tiles[dp // 2]
                    h = dp % 2
                    ps = mmps.tile([128, NQ], F32, tag="mm")
                    for dk in range(DP):
                        nc.tensor.matmul(ps[:], wqh_sl[:, h, dk], qrt_h[:, dk, :],
                                         start=(dk == 0), stop=(dk == DP - 1))
                    if h == 1:
                        wq_tiles.pop(dp // 2)
                    if h == 0 and dp // 2 + 3 < DP // 2:
                        load_wq2(dp // 2 + 3)
                    # window prefetch AFTER the last wq trigger (dp==11's
                    # load_wq2(7... is issued at dp==10) so the 2MB window
                    # transfers don't delay wq data the PE needs sooner;
                    # w0/w1 still land well before S starts (~67us)
                    if dp == 11:
                        load_window(0)
                    elif dp == 13:
                        load_window(1)
                    nc.vector.tensor_copy(qth[dp][:], ps[:])

            # ---------------- Phase S: scores + softmax + P^T
            with (
                tc.tile_pool(name="s", bufs=1) as sp,
                tc.tile_pool(name="p", bufs=2) as pp,
            ):
                s_t = [sp.tile([128, exts[t]], F32, tag=f"s{t}", name=f"s{t}") for t in range(B)]

                def softmax_t(t):
                    ext = exts[t]
                    nc.vector.reduce_max(
                        out=negmax[t][:], in_=rmax[t][:, :ext // JW],
                        axis=mybir.AxisListType.X, negate=True)
                    # descale: bias for exp must be -rowmax = -rowmax256/256
                    nc.vector.tensor_scalar_mul(
                        negmax[t][:], negmax[t][:], 1.0 / LSCALE)
                    for w2 in range(ext // JW):
                        pwin = pp.tile([128, JW], F16, tag=f"p{t}", name=f"p{t}")
                        rp = smallp.tile([128, 1], F32, tag=f"rsp{t}_{w2}", name=f"rsp{t}_{w2}")
                        rspart[(t, w2)] = rp
                        nc.scalar.activation(
                            pwin[:], s_t[t][:, w2 * JW:(w2 + 1) * JW],
                            mybir.ActivationFunctionType.Exp,
                            bias=negmax[t][:], scale=1.0 / LSCALE, accum_out=rp[:])
                        for jj in range(JW // 128):
                            jt = w2 * (JW // 128) + jj
                            trp = trps.tile([128, 128], F16, tag="tr")
                            nc.tensor.transpose(
                                trp[:], pwin[:, jj * 128:(jj + 1) * 128], ident[:])
                            nc.vector.tensor_copy(
                                pt[jt][:, t * 128:(t + 1) * 128], trp[:])
                    # rowsum = sum of window partials; recip
                    nc.vector.tensor_copy(rsum[t][:], rspart[(t, 0)][:])
                    for w2 in range(1, ext // JW):
                        nc.vector.tensor_add(
                            rsum[t][:], rsum[t][:], rspart[(t, w2)][:])
                    nc.vector.reciprocal(recip[t][:], rsum[t][:])

                for w in range(NW):
                    if w + 2 < NW:
                        load_window(w + 2)
                    # ev value-tile prefetch spread over mid-S iterations so
                    # it does not collide with the tail windows' loads
                    if 2 <= w <= 6:
                        load_ev(w - 2)
                    ewh = ew_tiles.pop(w)
                    for t in range(B):
                        if exts[t] <= JW * w:
                            continue
                        ps = mmps.tile([128, JW], F32, tag="mm")
                        for dp in range(DP):
                            nc.tensor.matmul(ps[:], qth[dp][:, t * 128:(t + 1) * 128],
                                             ewh[:, dp],
                                             start=(dp == 0), stop=(dp == DP - 1))
                        # copy scores to SBUF, folding in the causal mask on
                        # the last MASKW columns; track per-window row max
                        nmw = MASKW // JW
                        wloc = exts[t] // JW - 1 - w   # windows from the end
                        if wloc < nmw:
                            moff = (nmw - 1 - wloc) * JW
                            nc.vector.tensor_add(
                                s_t[t][:, w * JW:(w + 1) * JW], ps[:],
                                mask_sb[:, moff:moff + JW])
                        else:
                            nc.vector.tensor_copy(
                                s_t[t][:, w * JW:(w + 1) * JW], ps[:])
                        nc.vector.reduce_max(
                            out=rmax[t][:, w:w + 1],
                            in_=s_t[t][:, w * JW:(w + 1) * JW],
                            axis=mybir.AxisListType.X)
                        if JW * (w + 1) == exts[t]:
                            softmax_t(t)

            ewp_cm.__exit__(None, None, None)

            # ---------------- Phase PV: OpT[d, q] = sum_j E[j,d] P^T[j,q]
            with (
                tc.tile_pool(name="opt", bufs=1) as optp,
                tc.tile_pool(name="ovs", bufs=2) as ovp,
                tc.tile_pool(name="osb", bufs=2) as osbp,
            ):
                ov_tiles = {}

                def load_ov(dc):
                    ovs = ovp.tile([128, DP, JW], F16, tag="ovs", name="ovs")
                    nc.scalar.dma_start(ovs[:], ov_d[dc])
                    ov_tiles[dc] = ovs

                opt = [optp.tile([128, NQ], F16, tag=f"opt{dt}", name=f"opt{dt}") for dt in range(DP)]
                NPRE = min(5, DP)   # tiles 0..4 already prefetched mid-S
                for dt in range(DP):
                    evs = ev_tiles.pop(dt)
                    ps = pvps.tile([128, NQ], F32, tag="pv")
                    for jt in range(NJT):
                        nw_ = n_jt(jt)
                        nc.tensor.matmul(ps[:, :nw_], evs[:, jt], pt[jt][:, :nw_],
                                         start=(jt == 0), stop=(jt == NJT - 1))
                    if dt + NPRE < DP:
                        load_ev(dt + NPRE)
                    elif dt == max(0, DP - NPRE):
                        load_ov(0)
                    elif dt == max(1, DP - NPRE + 1):
                        load_ov(1)
                    nc.vector.tensor_copy(opt[dt][:], ps[:])

                # ---------------- Phase O: out = (OpT^T @ ov) * recip
                if True:
                    for dc in range(NDC):
                        if dc + 2 < NDC:
                            load_ov(dc + 2)
                        ovs = ov_tiles.pop(dc)
                        for t in range(B):
                            ps = mmps.tile([128, JW], F32, tag="mm")
                            for dt in range(DP):
                                nc.tensor.matmul(
                                    ps[:], opt[dt][:, t * 128:(t + 1) * 128],
                                    ovs[:, dt],
                                    start=(dt == 0), stop=(dt == DP - 1))
                            osb = osbp.tile([128, JW], F16, tag="osb")
                            nc.vector.tensor_scalar_mul(osb[:], ps[:], recip[t][:])
                            nc.sync.dma_start(
                                out_d[t * 128:(t + 1) * 128,
                                      dc * JW:(dc + 1) * JW], osb[:])

            evp_cm.__exit__(None, None, None)

    nc.compile()
    return nc


def make_in_maps(embedding, qk, ov, C=N_CORES, B=B_BANDS):
    """Host-side layout/dtype prep. Returns (in_maps, gtiles_per_core)."""
    N, D = embedding.shape
    DP = D // 128
    DPP = DP // 2
    JW = min(512, QT * C)
    NW = N // JW
    NJT = N // 128
    NQ = B * QT
    NDC = D // JW
    MASKW = QT * C

    E = np.ascontiguousarray(embedding.astype(np.float32))
    ET = np.ascontiguousarray(E.T)
    Eh = E.astype(np.float16)
    ETh = np.ascontiguousarray(ET.astype(np.float16))
    W = qk.astype(np.float32)
    Wh = W.astype(np.float16)
    # 256*Wh is exact in fp16 (|W| < 1 so |256*Wh| < 65504)
    Wh256 = (LSCALE * Wh.astype(np.float32)).astype(np.float16)
    OVh = ov.astype(np.float16)

    eth_t = np.ascontiguousarray(
        ETh.reshape(DP, 128, NW, JW).transpose(2, 1, 0, 3))
    # paired layout: wqh[c2, b, j, a, d] = 256*Wh[a*128+b, (2*c2+j)*128+d]
    wqh_t = np.ascontiguousarray(
        Wh256.reshape(DP, 128, DPP, 2, 128).transpose(2, 1, 3, 0, 4))
    ev_t = np.ascontiguousarray(
        Eh.reshape(NJT, 128, DP, 128).transpose(2, 1, 0, 3))
    ov_t = np.ascontiguousarray(
        OVh.reshape(DP, 128, NDC, JW).transpose(2, 1, 0, 3))

    r = np.arange(128)[:, None]
    m = np.arange(MASKW)[None, :]

    in_maps = []
    gtiles_all = []
    for i in range(C):
        gtiles = [C * (B - 1 - t) + i for t in range(B)]
        gtiles_all.append(gtiles)
        qrh = np.concatenate(
            [ETh[:, 128 * g:128 * (g + 1)] for g in gtiles], axis=1)
        # partition-major for fat contiguous DMA runs:
        # qrth[b, dk, q] = ETh_own[dk*128+b, q]
        qrth = qrh.reshape(DP, 128, NQ).transpose(1, 0, 2)
        mask = np.where(m <= 128 * i + r, 0.0, MASK_NEG).astype(np.float32)
        in_maps.append({
            "qrth": np.ascontiguousarray(qrth),
            "wqh": wqh_t,
            "eth": eth_t,
            "ev": ev_t, "ov": ov_t,
            "mask": mask,
        })
    return in_maps, gtiles_all


_CACHED = {}


def kernel(embedding, qk, ov):
    from concourse.bass_utils import run_bass_kernel_spmd

    key = "main"
    if key not in _CACHED:
        _CACHED[key] = build_program()
    nc = _CACHED[key]

    in_maps, gtiles_all = make_in_maps(embedding, qk, ov)
    res = run_bass_kernel_spmd(nc, in_maps, core_ids=list(range(N_CORES)))

    N, D = embedding.shape
    out = np.empty((N, D), dtype=np.float32)
    for i in range(N_CORES):
        o = np.asarray(res.results[i]["out"], dtype=np.float32)
        for t, g in enumerate(gtiles_all[i]):
            out[128 * g:128 * (g + 1)] = o[128 * t:128 * (t + 1)]
    return out
